# revision 7
# baseline (speedup 1.0000x reference)
"""Trainium2 Bass kernel for AsymmetricWeightsDequantizer.

result = zero_point + weight * scale  (per [O, G] group, broadcast over GS)
         + svd_up @ svd_down          (rank-128 correction)

Sharding: output dim O split across 8 cores (1024 rows each), svd_down
replicated. Per core:
  - PE:  psum = svd_upT(bf16) @ svd_down(bf16)          [rank-128 term]
              + [z_hi; z_lo](bf16) @ [E; E](bf16)       [exact zero_point
         via hi/lo bf16 split against a 0/1 group-indicator matrix]
  - DVE: fused scalar_tensor_tensor out = (w * scale) + psum for part of
         the group-chunks, plus one batched tensor_tensor add for the
         chunks ACT dequantized
  - ACT: activation(Copy, scale) dequant (w * scale) for the rest
  - weights repacked host-side to uint8 (values are 0..255) to cut HBM
    read traffic 4x
"""

import numpy as np
import ml_dtypes

import concourse.bass as bass
import concourse.bacc as bacc
import concourse.mybir as mybir
import concourse.tile as tile
from concourse import bass_utils

O, G, GS = 8192, 64, 128
I = G * GS              # 8192
RANK = 128
NCORES = 8
OP = O // NCORES        # 1024 rows per core
NT = OP // 128          # 8 partition tiles per core
NBLK = 4                # 2048-col blocks per row tile
BLK = I // NBLK         # 2048
GPB = G // NBLK         # 16 groups per block
NPS = BLK // 512        # 512-col matmul slices per block
# per 16-chunk block: j < DVE_FUSED fused on DVE (reads PSUM directly);
# next GPS_SHARE chunks dequant on GpSimd; rest dequant on ACT.  All
# non-fused chunks get their PSUM term via one batched DVE tensor_tensor.
DVE_FUSED = 0
GPS_SHARE = 8

BF16 = ml_dtypes.bfloat16
F32 = mybir.dt.float32
U8 = mybir.dt.uint8

_cached_nc = None


def _build():
    global _cached_nc
    if _cached_nc is not None:
        return _cached_nc

    nc = bacc.Bacc("TRN2", target_bir_lowering=False, debug=False,
                   num_devices=NCORES)

    w_d = nc.dram_tensor("w", [OP, I], U8, kind="ExternalInput")
    sc_d = nc.dram_tensor("scale_r", [128, NT * G], F32, kind="ExternalInput")
    zp_d = nc.dram_tensor("zeroT2", [128, OP], mybir.dt.bfloat16,
                          kind="ExternalInput")
    eb_d = nc.dram_tensor("eblk", [128, I], mybir.dt.bfloat16,
                          kind="ExternalInput")
    up_d = nc.dram_tensor("upT", [RANK, OP], mybir.dt.bfloat16,
                          kind="ExternalInput")
    dn_d = nc.dram_tensor("down", [RANK, I], mybir.dt.bfloat16,
                          kind="ExternalInput")
    out_d = nc.dram_tensor("out", [OP, I], F32, kind="ExternalOutput")

    with tile.TileContext(nc) as tc:
        with (
            tc.tile_pool(name="const", bufs=1) as cpool,
            tc.tile_pool(name="wp", bufs=3) as wpool,
            tc.tile_pool(name="outp", bufs=4) as opool,
            tc.tile_pool(name="ps", bufs=2, space="PSUM") as pspool,
        ):
            down_sb = cpool.tile([RANK, I], mybir.dt.bfloat16)
            upT_sb = cpool.tile([RANK, OP], mybir.dt.bfloat16)
            eb_sb = cpool.tile([128, I], mybir.dt.bfloat16)
            zp_sb = cpool.tile([128, OP], mybir.dt.bfloat16)
            sc_sb = cpool.tile([128, NT * G], F32)
            # chunk the big constants so the first matmuls / fused ops are
            # gated on ~1 MiB of DMA, not the full 4.75 MiB
            nc.sync.dma_start(upT_sb[:], up_d[:])
            nc.sync.dma_start(down_sb[:, 0:BLK], dn_d[:, 0:BLK])
            nc.sync.dma_start(sc_sb[:], sc_d[:])
            nc.sync.dma_start(zp_sb[:], zp_d[:])
            nc.sync.dma_start(eb_sb[:, 0:BLK], eb_d[:, 0:BLK])
            for nb in range(1, NBLK):
                s = slice(nb * BLK, (nb + 1) * BLK)
                nc.sync.dma_start(down_sb[:, s], dn_d[:, s])
                nc.sync.dma_start(eb_sb[:, s], eb_d[:, s])

            for t in range(NT):
                w_sb = wpool.tile([128, I], U8)
                # SWDGE: keeps weight prefetch off the SP HWDGE ring so it
                # can't queue behind output stores
                nc.gpsimd.dma_start(w_sb[:], w_d[t * 128:(t + 1) * 128, :])

                for nb in range(NBLK):
                    ps = pspool.tile([128, BLK], F32)
                    # svd term: same stationary weights for all 4 banks
                    for q in range(NPS):
                        n = nb * NPS + q
                        nc.tensor.matmul(
                            ps[:, q * 512:(q + 1) * 512],
                            upT_sb[:, t * 128:(t + 1) * 128],
                            down_sb[:, n * 512:(n + 1) * 512],
                            start=True, stop=False,
                        )
                    # zero_point term (exact via bf16 hi/lo pair)
                    for q in range(NPS):
                        n = nb * NPS + q
                        nc.tensor.matmul(
                            ps[:, q * 512:(q + 1) * 512],
                            zp_sb[:, t * 128:(t + 1) * 128],
                            eb_sb[:, n * 512:(n + 1) * 512],
                            start=False, stop=True,
                        )

                    out_sb = opool.tile([128, BLK], F32)
                    for j in range(GPB):
                        g = nb * GPB + j
                        col = t * G + g
                        dst = out_sb[:, j * GS:(j + 1) * GS]
                        src = w_sb[:, g * GS:(g + 1) * GS]
                        s_ap = sc_sb[:, col:col + 1]
                        if j < DVE_FUSED:
                            nc.vector.scalar_tensor_tensor(
                                dst, src, s_ap, ps[:, j * GS:(j + 1) * GS],
                                mybir.AluOpType.mult, mybir.AluOpType.add)
                        elif j < DVE_FUSED + GPS_SHARE:
                            nc.gpsimd.tensor_scalar_mul(dst, src, s_ap)
                        else:
                            nc.scalar.activation(
                                dst, src, mybir.ActivationFunctionType.Copy,
                                bias=0.0, scale=s_ap)
                    if DVE_FUSED < GPB:
                        tail = slice(DVE_FUSED * GS, GPB * GS)
                        nc.vector.tensor_tensor(
                            out_sb[:, tail], out_sb[:, tail], ps[:, tail],
                            op=mybir.AluOpType.add)

                    nc.sync.dma_start(
                        out_d[t * 128:(t + 1) * 128, nb * BLK:(nb + 1) * BLK],
                        out_sb[:])

    nc.compile()
    _cached_nc = nc
    return nc


def _make_in_maps(weight, scale, zero_point, svd_up, svd_down):
    w = np.ascontiguousarray(weight.reshape(O, I)).astype(np.uint8)
    sc = np.ascontiguousarray(scale.reshape(O, G).astype(np.float32))
    zp = np.ascontiguousarray(zero_point.reshape(O, G).astype(np.float32))
    down_b = np.ascontiguousarray(svd_down).astype(BF16)

    # group-indicator matrix, stacked twice for the hi/lo zero split
    eblk = np.zeros((128, I), dtype=BF16)
    for g in range(G):
        eblk[g, g * GS:(g + 1) * GS] = 1
        eblk[G + g, g * GS:(g + 1) * GS] = 1

    in_maps = []
    for c in range(NCORES):
        sl = slice(c * OP, (c + 1) * OP)
        scr = np.ascontiguousarray(
            sc[sl].reshape(NT, 128, G).transpose(1, 0, 2).reshape(128, NT * G))
        z = zp[sl]                           # [OP, G] f32
        z_hi = z.astype(BF16)
        z_lo = (z - z_hi.astype(np.float32)).astype(BF16)
        zeroT2 = np.concatenate([z_hi.T, z_lo.T], axis=0)  # [128, OP] bf16
        upT = np.ascontiguousarray(svd_up[sl].T).astype(BF16)
        in_maps.append({
            "w": np.ascontiguousarray(w[sl]),
            "scale_r": scr,
            "zeroT2": np.ascontiguousarray(zeroT2),
            "eblk": eblk,
            "upT": upT,
            "down": down_b,
        })
    return in_maps


def _run(in_maps, trace=False, **kwargs):
    nc = _build()
    return bass_utils.run_bass_kernel_spmd(
        nc, in_maps, core_ids=list(range(NCORES)), trace=trace, **kwargs)


def kernel(weight, scale, zero_point, svd_up, svd_down):
    in_maps = _make_in_maps(np.asarray(weight), np.asarray(scale),
                            np.asarray(zero_point), np.asarray(svd_up),
                            np.asarray(svd_down))
    res = _run(in_maps)
    return np.concatenate([res.results[c]["out"] for c in range(NCORES)],
                          axis=0)


# revision 8
# speedup vs baseline: 3.0689x; 3.0689x over previous
"""Trainium2 Bass kernel for AsymmetricWeightsDequantizer.

result = zero_point + weight * scale  (per [O, G] group, broadcast over GS)
         + svd_up @ svd_down          (rank-128 correction)

Sharding: output dim O split across 8 cores (1024 rows each), svd_down
replicated. Per core:
  - PE:  psum = svd_upT(bf16) @ svd_down(bf16)          [rank-128 term]
              + [z_hi; z_lo](bf16) @ [E; E](bf16)       [exact zero_point
         via hi/lo bf16 split against a 0/1 group-indicator matrix]
  - DVE: fused scalar_tensor_tensor out = (w * scale) + psum for part of
         the group-chunks, plus one batched tensor_tensor add for the
         chunks ACT dequantized
  - ACT: activation(Copy, scale) dequant (w * scale) for the rest
  - weights repacked host-side to uint8 (values are 0..255) to cut HBM
    read traffic 4x
"""

import numpy as np
import ml_dtypes

import concourse.bass as bass
import concourse.bacc as bacc
import concourse.mybir as mybir
import concourse.tile as tile
from concourse import bass_utils

O, G, GS = 8192, 64, 128
I = G * GS              # 8192
RANK = 128
NCORES = 8
OP = O // NCORES        # 1024 rows per core
NT = OP // 128          # 8 partition tiles per core
NBLK = 4                # 2048-col blocks per row tile
BLK = I // NBLK         # 2048
GPB = G // NBLK         # 16 groups per block
NPS = BLK // 512        # 512-col matmul slices per block
# per 16-chunk block: j < DVE_FUSED fused on DVE (reads PSUM directly);
# next GPS_SHARE chunks dequant on GpSimd; rest dequant on ACT.  All
# non-fused chunks get their PSUM term via one batched DVE tensor_tensor.
DVE_FUSED = 8           # GpSimd elementwise measured ~2 us/chunk: unusable
GPS_SHARE = 0

BF16 = ml_dtypes.bfloat16
F32 = mybir.dt.float32
U8 = mybir.dt.uint8

_cached_nc = None


def _build():
    global _cached_nc
    if _cached_nc is not None:
        return _cached_nc

    nc = bacc.Bacc("TRN2", target_bir_lowering=False, debug=False,
                   num_devices=NCORES)

    w_d = nc.dram_tensor("w", [OP, I], U8, kind="ExternalInput")
    sc_d = nc.dram_tensor("scale_r", [128, NT * G], F32, kind="ExternalInput")
    zp_d = nc.dram_tensor("zeroT2", [128, OP], mybir.dt.bfloat16,
                          kind="ExternalInput")
    eb_d = nc.dram_tensor("eblk", [128, I], mybir.dt.bfloat16,
                          kind="ExternalInput")
    up_d = nc.dram_tensor("upT", [RANK, OP], mybir.dt.bfloat16,
                          kind="ExternalInput")
    dn_d = nc.dram_tensor("down", [RANK, I], mybir.dt.bfloat16,
                          kind="ExternalInput")
    out_d = nc.dram_tensor("out", [OP, I], F32, kind="ExternalOutput")

    with tile.TileContext(nc) as tc:
        with (
            tc.tile_pool(name="const", bufs=1) as cpool,
            tc.tile_pool(name="wp", bufs=3) as wpool,
            tc.tile_pool(name="outp", bufs=4) as opool,
            tc.tile_pool(name="ps", bufs=2, space="PSUM") as pspool,
        ):
            down_sb = cpool.tile([RANK, I], mybir.dt.bfloat16)
            upT_sb = cpool.tile([RANK, OP], mybir.dt.bfloat16)
            eb_sb = cpool.tile([128, I], mybir.dt.bfloat16)
            zp_sb = cpool.tile([128, OP], mybir.dt.bfloat16)
            sc_sb = cpool.tile([128, NT * G], F32)
            # chunk the big constants so the first matmuls / fused ops are
            # gated on ~1 MiB of DMA, not the full 4.75 MiB
            nc.sync.dma_start(upT_sb[:], up_d[:])
            nc.sync.dma_start(down_sb[:, 0:BLK], dn_d[:, 0:BLK])
            nc.sync.dma_start(sc_sb[:], sc_d[:])
            nc.sync.dma_start(zp_sb[:], zp_d[:])
            nc.sync.dma_start(eb_sb[:, 0:BLK], eb_d[:, 0:BLK])
            for nb in range(1, NBLK):
                s = slice(nb * BLK, (nb + 1) * BLK)
                nc.sync.dma_start(down_sb[:, s], dn_d[:, s])
                nc.sync.dma_start(eb_sb[:, s], eb_d[:, s])

            for t in range(NT):
                w_sb = wpool.tile([128, I], U8)
                # SWDGE: keeps weight prefetch off the SP HWDGE ring so it
                # can't queue behind output stores
                nc.gpsimd.dma_start(w_sb[:], w_d[t * 128:(t + 1) * 128, :])

                for nb in range(NBLK):
                    ps = pspool.tile([128, BLK], F32)
                    # svd term: same stationary weights for all 4 banks
                    for q in range(NPS):
                        n = nb * NPS + q
                        nc.tensor.matmul(
                            ps[:, q * 512:(q + 1) * 512],
                            upT_sb[:, t * 128:(t + 1) * 128],
                            down_sb[:, n * 512:(n + 1) * 512],
                            start=True, stop=False,
                        )
                    # zero_point term (exact via bf16 hi/lo pair)
                    for q in range(NPS):
                        n = nb * NPS + q
                        nc.tensor.matmul(
                            ps[:, q * 512:(q + 1) * 512],
                            zp_sb[:, t * 128:(t + 1) * 128],
                            eb_sb[:, n * 512:(n + 1) * 512],
                            start=False, stop=True,
                        )

                    out_sb = opool.tile([128, BLK], F32)
                    for j in range(GPB):
                        g = nb * GPB + j
                        col = t * G + g
                        dst = out_sb[:, j * GS:(j + 1) * GS]
                        src = w_sb[:, g * GS:(g + 1) * GS]
                        s_ap = sc_sb[:, col:col + 1]
                        if j < DVE_FUSED:
                            nc.vector.scalar_tensor_tensor(
                                dst, src, s_ap, ps[:, j * GS:(j + 1) * GS],
                                mybir.AluOpType.mult, mybir.AluOpType.add)
                        elif j < DVE_FUSED + GPS_SHARE:
                            nc.gpsimd.tensor_scalar_mul(dst, src, s_ap)
                        else:
                            nc.scalar.activation(
                                dst, src, mybir.ActivationFunctionType.Copy,
                                bias=0.0, scale=s_ap)
                    if DVE_FUSED < GPB:
                        tail = slice(DVE_FUSED * GS, GPB * GS)
                        nc.vector.tensor_tensor(
                            out_sb[:, tail], out_sb[:, tail], ps[:, tail],
                            op=mybir.AluOpType.add)

                    nc.sync.dma_start(
                        out_d[t * 128:(t + 1) * 128, nb * BLK:(nb + 1) * BLK],
                        out_sb[:])

    nc.compile()
    _cached_nc = nc
    return nc


def _make_in_maps(weight, scale, zero_point, svd_up, svd_down):
    w = np.ascontiguousarray(weight.reshape(O, I)).astype(np.uint8)
    sc = np.ascontiguousarray(scale.reshape(O, G).astype(np.float32))
    zp = np.ascontiguousarray(zero_point.reshape(O, G).astype(np.float32))
    down_b = np.ascontiguousarray(svd_down).astype(BF16)

    # group-indicator matrix, stacked twice for the hi/lo zero split
    eblk = np.zeros((128, I), dtype=BF16)
    for g in range(G):
        eblk[g, g * GS:(g + 1) * GS] = 1
        eblk[G + g, g * GS:(g + 1) * GS] = 1

    in_maps = []
    for c in range(NCORES):
        sl = slice(c * OP, (c + 1) * OP)
        scr = np.ascontiguousarray(
            sc[sl].reshape(NT, 128, G).transpose(1, 0, 2).reshape(128, NT * G))
        z = zp[sl]                           # [OP, G] f32
        z_hi = z.astype(BF16)
        z_lo = (z - z_hi.astype(np.float32)).astype(BF16)
        zeroT2 = np.concatenate([z_hi.T, z_lo.T], axis=0)  # [128, OP] bf16
        upT = np.ascontiguousarray(svd_up[sl].T).astype(BF16)
        in_maps.append({
            "w": np.ascontiguousarray(w[sl]),
            "scale_r": scr,
            "zeroT2": np.ascontiguousarray(zeroT2),
            "eblk": eblk,
            "upT": upT,
            "down": down_b,
        })
    return in_maps


def _run(in_maps, trace=False, **kwargs):
    nc = _build()
    return bass_utils.run_bass_kernel_spmd(
        nc, in_maps, core_ids=list(range(NCORES)), trace=trace, **kwargs)


def kernel(weight, scale, zero_point, svd_up, svd_down):
    in_maps = _make_in_maps(np.asarray(weight), np.asarray(scale),
                            np.asarray(zero_point), np.asarray(svd_up),
                            np.asarray(svd_down))
    res = _run(in_maps)
    return np.concatenate([res.results[c]["out"] for c in range(NCORES)],
                          axis=0)


# revision 10
# speedup vs baseline: 3.3630x; 1.0958x over previous
"""Trainium2 Bass kernel for AsymmetricWeightsDequantizer.

result = zero_point + weight * scale  (per [O, G] group, broadcast over GS)
         + svd_up @ svd_down          (rank-128 correction)

Sharding: output dim O split across 8 cores (1024 rows each), svd_down
replicated. Per core:
  - PE:  psum = svd_upT(bf16) @ svd_down(bf16)          [rank-128 term]
              + [z_hi; z_lo](bf16) @ [E; E](bf16)       [exact zero_point
         via hi/lo bf16 split against a 0/1 group-indicator matrix]
  - DVE: fused scalar_tensor_tensor out = (w * scale) + psum for part of
         the group-chunks, plus one batched tensor_tensor add for the
         chunks ACT dequantized
  - ACT: activation(Copy, scale) dequant (w * scale) for the rest
  - weights repacked host-side to uint8 (values are 0..255) to cut HBM
    read traffic 4x
"""

import numpy as np
import ml_dtypes

import concourse.bass as bass
import concourse.bacc as bacc
import concourse.mybir as mybir
import concourse.tile as tile
from concourse import bass_utils

O, G, GS = 8192, 64, 128
I = G * GS              # 8192
RANK = 128
NCORES = 8
OP = O // NCORES        # 1024 rows per core
NT = OP // 128          # 8 partition tiles per core
NBLK = 4                # 2048-col blocks per row tile
BLK = I // NBLK         # 2048
GPB = G // NBLK         # 16 groups per block
NPS = BLK // 512        # 512-col matmul slices per block
# per 16-chunk block: j < DVE_FUSED fused on DVE (reads PSUM directly);
# next GPS_SHARE chunks dequant on GpSimd; rest dequant on ACT.  All
# non-fused chunks get their PSUM term via one batched DVE tensor_tensor.
DVE_FUSED = 7           # GpSimd elementwise measured ~2 us/chunk: unusable
GPS_SHARE = 0

BF16 = ml_dtypes.bfloat16
F32 = mybir.dt.float32
U8 = mybir.dt.uint8

_cached_nc = None


def _build():
    global _cached_nc
    if _cached_nc is not None:
        return _cached_nc

    nc = bacc.Bacc("TRN2", target_bir_lowering=False, debug=False,
                   num_devices=NCORES)

    w_d = nc.dram_tensor("w", [OP, I], U8, kind="ExternalInput")
    sc_d = nc.dram_tensor("scale_r", [128, NT * G], F32, kind="ExternalInput")
    zp_d = nc.dram_tensor("zeroT2", [128, OP], mybir.dt.bfloat16,
                          kind="ExternalInput")
    eb_d = nc.dram_tensor("eblk", [128, I], mybir.dt.bfloat16,
                          kind="ExternalInput")
    up_d = nc.dram_tensor("upT", [RANK, OP], mybir.dt.bfloat16,
                          kind="ExternalInput")
    dn_d = nc.dram_tensor("down", [RANK, I], mybir.dt.bfloat16,
                          kind="ExternalInput")
    out_d = nc.dram_tensor("out", [OP, I], F32, kind="ExternalOutput")

    with tile.TileContext(nc) as tc:
        with (
            tc.tile_pool(name="const", bufs=1) as cpool,
            tc.tile_pool(name="wp", bufs=3) as wpool,
            tc.tile_pool(name="outp", bufs=4) as opool,
            tc.tile_pool(name="ps", bufs=2, space="PSUM") as pspool,
        ):
            down_sb = cpool.tile([RANK, I], mybir.dt.bfloat16)
            upT_sb = cpool.tile([RANK, OP], mybir.dt.bfloat16)
            eb_sb = cpool.tile([128, I], mybir.dt.bfloat16)
            zp_sb = cpool.tile([128, OP], mybir.dt.bfloat16)
            sc_sb = cpool.tile([128, NT * G], F32)
            # constants split across the two HWDGE rings (SP + ACT-queue,
            # idle at kernel start) and chunked so the first block's
            # matmuls are gated on <1 MiB of DMA each
            nc.sync.dma_start(upT_sb[:], up_d[:])
            nc.sync.dma_start(down_sb[:, 0:BLK], dn_d[:, 0:BLK])
            nc.scalar.dma_start(sc_sb[:], sc_d[:])
            nc.scalar.dma_start(zp_sb[:], zp_d[:])
            nc.scalar.dma_start(eb_sb[:, 0:BLK], eb_d[:, 0:BLK])
            for nb in range(1, NBLK):
                s = slice(nb * BLK, (nb + 1) * BLK)
                nc.sync.dma_start(down_sb[:, s], dn_d[:, s])
                nc.scalar.dma_start(eb_sb[:, s], eb_d[:, s])

            for t in range(NT):
                w_sb = wpool.tile([128, I], U8)
                nc.sync.dma_start(w_sb[:], w_d[t * 128:(t + 1) * 128, :])

                for nb in range(NBLK):
                    ps = pspool.tile([128, BLK], F32)
                    # svd term: same stationary weights for all 4 banks
                    for q in range(NPS):
                        n = nb * NPS + q
                        nc.tensor.matmul(
                            ps[:, q * 512:(q + 1) * 512],
                            upT_sb[:, t * 128:(t + 1) * 128],
                            down_sb[:, n * 512:(n + 1) * 512],
                            start=True, stop=False,
                        )
                    # zero_point term (exact via bf16 hi/lo pair)
                    for q in range(NPS):
                        n = nb * NPS + q
                        nc.tensor.matmul(
                            ps[:, q * 512:(q + 1) * 512],
                            zp_sb[:, t * 128:(t + 1) * 128],
                            eb_sb[:, n * 512:(n + 1) * 512],
                            start=False, stop=True,
                        )

                    out_sb = opool.tile([128, BLK], F32)
                    for j in range(GPB):
                        g = nb * GPB + j
                        col = t * G + g
                        dst = out_sb[:, j * GS:(j + 1) * GS]
                        src = w_sb[:, g * GS:(g + 1) * GS]
                        s_ap = sc_sb[:, col:col + 1]
                        if j < DVE_FUSED:
                            nc.vector.scalar_tensor_tensor(
                                dst, src, s_ap, ps[:, j * GS:(j + 1) * GS],
                                mybir.AluOpType.mult, mybir.AluOpType.add)
                        elif j < DVE_FUSED + GPS_SHARE:
                            nc.gpsimd.tensor_scalar_mul(dst, src, s_ap)
                        else:
                            nc.scalar.activation(
                                dst, src, mybir.ActivationFunctionType.Copy,
                                bias=0.0, scale=s_ap)
                    if DVE_FUSED < GPB:
                        tail = slice(DVE_FUSED * GS, GPB * GS)
                        nc.vector.tensor_tensor(
                            out_sb[:, tail], out_sb[:, tail], ps[:, tail],
                            op=mybir.AluOpType.add)

                    nc.sync.dma_start(
                        out_d[t * 128:(t + 1) * 128, nb * BLK:(nb + 1) * BLK],
                        out_sb[:])

    nc.compile()
    _cached_nc = nc
    return nc


def _make_in_maps(weight, scale, zero_point, svd_up, svd_down):
    w = np.ascontiguousarray(weight.reshape(O, I)).astype(np.uint8)
    sc = np.ascontiguousarray(scale.reshape(O, G).astype(np.float32))
    zp = np.ascontiguousarray(zero_point.reshape(O, G).astype(np.float32))
    down_b = np.ascontiguousarray(svd_down).astype(BF16)

    # group-indicator matrix, stacked twice for the hi/lo zero split
    eblk = np.zeros((128, I), dtype=BF16)
    for g in range(G):
        eblk[g, g * GS:(g + 1) * GS] = 1
        eblk[G + g, g * GS:(g + 1) * GS] = 1

    in_maps = []
    for c in range(NCORES):
        sl = slice(c * OP, (c + 1) * OP)
        scr = np.ascontiguousarray(
            sc[sl].reshape(NT, 128, G).transpose(1, 0, 2).reshape(128, NT * G))
        z = zp[sl]                           # [OP, G] f32
        z_hi = z.astype(BF16)
        z_lo = (z - z_hi.astype(np.float32)).astype(BF16)
        zeroT2 = np.concatenate([z_hi.T, z_lo.T], axis=0)  # [128, OP] bf16
        upT = np.ascontiguousarray(svd_up[sl].T).astype(BF16)
        in_maps.append({
            "w": np.ascontiguousarray(w[sl]),
            "scale_r": scr,
            "zeroT2": np.ascontiguousarray(zeroT2),
            "eblk": eblk,
            "upT": upT,
            "down": down_b,
        })
    return in_maps


def _run(in_maps, trace=False, **kwargs):
    nc = _build()
    return bass_utils.run_bass_kernel_spmd(
        nc, in_maps, core_ids=list(range(NCORES)), trace=trace, **kwargs)


def kernel(weight, scale, zero_point, svd_up, svd_down):
    in_maps = _make_in_maps(np.asarray(weight), np.asarray(scale),
                            np.asarray(zero_point), np.asarray(svd_up),
                            np.asarray(svd_down))
    res = _run(in_maps)
    return np.concatenate([res.results[c]["out"] for c in range(NCORES)],
                          axis=0)
